# revision 36
# baseline (speedup 1.0000x reference)
"""Multi-head self-attention (B=1, S=4096, D=512, H=8) on 8 trn2 NeuronCores.

Sharding: one head per core (head/tensor parallel). Each core computes its
head's Q/K/V projections from the full (transposed) query, runs attention
without materializing the full score matrix (streaming over key chunks,
softmax denominator via a ones-row in the transposed V), applies its slice
of out_proj fused with softmax normalization, and writes an unnormalized
partial [S, D] output (fp16). Host sums the 8 partials and adds the folded
bias.

All matmul operands are fp16 (1 PE cycle/row); accumulation stays f32 in
PSUM. Bias handling exploits softmax invariance: the K bias cancels inside
softmax, and the V bias (+out bias) folds into a host-side additive vector,
so only the Q bias is applied on device.

Schedule: one global software pipeline over (group, key-batch) steps with
scores+exp emitted one step ahead of the AV accumulation, so the ACT engine
(exp) never stalls at group boundaries. K/V projections are woven
just-in-time into group 0's batches (chasing the qt DMA); V is projected as
V^T (cheap N=512 matmuls) and transposed to key-major via the DMA crossbar
(off the PE). Q projection for group g+1 and the out-proj of group g-1 are
woven into group g's batches.
"""

import sys

sys.path.insert(0, "/opt/trn_rl_repo")

import numpy as np

EMBED = 512
HEADS = 8
HD = 64          # head dim
S = 4096         # sequence length
P = 128          # partitions
NSK = S // P     # 32 key chunks of 128
QG = 512         # query group width (matmul free dim)
NQG = S // QG    # 8 query groups
NDC = EMBED // P # 4 contraction chunks for projections
SCALE = HD ** -0.5
EB = 3           # key chunks per exp batch (PSUM banks per slot)
NB = (NSK + EB - 1) // EB
VTP = 80         # vt_hd partition count (HD + ones row, padded to 16)

_compiled = {}
_last = {}


def _build(n_cores=8):
    import concourse.bacc as bacc
    import concourse.mybir as mybir
    import concourse.tile as tile

    f32 = mybir.dt.float32
    f16 = mybir.dt.float16

    nc = bacc.Bacc("TRN2", target_bir_lowering=False, debug=False,
                   num_devices=n_cores)

    qt = nc.dram_tensor("qt", [EMBED, S], f16, kind="ExternalInput")
    w_in = nc.dram_tensor("w_in", [EMBED, 3 * HD], f16, kind="ExternalInput")
    wo = nc.dram_tensor("wo", [HD, EMBED], f16, kind="ExternalInput")
    bq = nc.dram_tensor("bq", [HD, 1], f32, kind="ExternalInput")
    out_p = nc.dram_tensor("out_p", [S, EMBED], f16, kind="ExternalOutput")
    den7 = nc.dram_tensor("den7", [1, QG], f16, kind="ExternalOutput")

    with tile.TileContext(nc) as tc:
        _emit(tc, nc, mybir, qt, w_in, wo, bq, out_p, den7, f32, f16)

    nc.compile()
    return nc


def _emit(tc, nc, mybir, qt, w_in, wo, bq, out_p, den7, f32, f16):
    from contextlib import ExitStack

    Exp = mybir.ActivationFunctionType.Exp

    with ExitStack() as ctx:
        singles = ctx.enter_context(tc.tile_pool(name="singles", bufs=1))

        # warm up the ACT exp table while DMAs run
        warm = singles.tile([1, 1], f32, tag="warm")
        nc.vector.memset(warm, 0.0)
        warm2 = singles.tile([1, 1], f32, tag="warm2")
        nc.scalar.activation(warm2, warm, Exp)

        # --- weights first on the sync/gpsimd queues (they gate every
        # projection matmul; the scalar/ACT sequencer is busy loading the exp
        # table early on), then qt group 0, then the rest of qt. Each group's
        # contraction chunks land on different queues so a group becomes
        # usable as early as possible ---
        dqs = [nc.sync, nc.scalar, nc.gpsimd]
        qt_sb = [[None] * NQG for _ in range(NDC)]

        def qt_tile(c, h):
            tl = singles.tile([P, QG], f16, tag=f"qt{c}_{h}", name=f"qt{c}_{h}")
            qt_sb[c][h] = tl
            return tl

        win_sb = singles.tile([P, NDC, 3 * HD], f16, tag="win")
        wo_sb = singles.tile([HD, EMBED], f16, tag="wo")
        bq_sb = singles.tile([HD, 1], f32, tag="bq")

        # pair each contraction chunk's weights with its qt slice, in the
        # order the first K/Q/V chains consume them; wo/bq (needed much
        # later) go last
        nc.sync.dma_start(out=win_sb[:, 0, :], in_=w_in[0:P, :])
        nc.sync.dma_start(out=qt_tile(0, 0), in_=qt[0:P, 0:QG])
        nc.gpsimd.dma_start(out=win_sb[:, 1, :], in_=w_in[P:2 * P, :])
        nc.gpsimd.dma_start(out=qt_tile(1, 0), in_=qt[P:2 * P, 0:QG])
        nc.sync.dma_start(out=win_sb[:, 2, :], in_=w_in[2 * P:3 * P, :])
        nc.gpsimd.dma_start(out=qt_tile(2, 0), in_=qt[2 * P:3 * P, 0:QG])
        nc.sync.dma_start(out=qt_tile(3, 0), in_=qt[3 * P:4 * P, 0:QG])
        nc.gpsimd.dma_start(out=win_sb[:, 3, :], in_=w_in[3 * P:4 * P, :])
        nc.gpsimd.dma_start(out=bq_sb, in_=bq[:, :])
        nc.sync.dma_start(out=wo_sb, in_=wo[:, :])

        # rest of qt; the scalar queue idles during the exp-table load, so it
        # leads the rotation for group 1 onward
        rot = [nc.scalar, nc.sync, nc.gpsimd]
        qi = 0
        for h in range(1, NQG):
            for c in range(NDC):
                rot[qi % len(rot)].dma_start(
                    out=qt_tile(c, h),
                    in_=qt[c * P:(c + 1) * P, h * QG:(h + 1) * QG])
                qi += 1

        def qts(c, col0, col1):
            h = col0 // QG
            assert col1 <= (h + 1) * QG
            return qt_sb[c][h][:, col0 - h * QG:col1 - h * QG]

        # persistent activations
        k_sb = singles.tile([HD, S], f16, tag="k")        # K^T per head
        q_sb = singles.tile([HD, S], f16, tag="q")        # Q^T per head
        ot_sb = singles.tile([HD + 1, S], f16, tag="ot")  # out^T + denom row
        vt_hd = singles.tile([VTP, S], f16, tag="vthd")   # V^T + ones row
        vtr = [singles.tile([P, 4, VTP], f16, tag=f"vtr{h}", name=f"vtr{h}")
               for h in range(NQG)]                       # V key-major chunks
        den_all = singles.tile([P, NSK], f16, tag="den")
        recip_all = singles.tile([P, NSK], f32, tag="recip")

        with ExitStack() as cctx:
            s_pool = cctx.enter_context(
                tc.tile_pool(name="s_pool", bufs=2, space="PSUM"))
            acc_pool = cctx.enter_context(
                tc.tile_pool(name="acc_pool", bufs=1, space="PSUM"))
            oq_pool = cctx.enter_context(
                tc.tile_pool(name="oq_pool", bufs=1, space="PSUM"))
            p_pool = cctx.enter_context(tc.tile_pool(name="p_pool", bufs=5))
            o_pool = cctx.enter_context(tc.tile_pool(name="o_pool", bufs=3))

            batches = [list(range(b * EB, min((b + 1) * EB, NSK)))
                       for b in range(NB)]
            den_qs = [nc.gpsimd, nc.sync, nc.scalar, nc.gpsimd]

            def emit_k(h):
                sl = slice(h * QG, (h + 1) * QG)
                kacc = oq_pool.tile([P, EMBED], f32, tag="oq", name="kacc")
                for c in range(NDC):
                    nc.tensor.matmul(kacc[0:HD, 0:QG], win_sb[:, c, HD:2 * HD],
                                     qts(c, h * QG, (h + 1) * QG),
                                     start=(c == 0), stop=(c == NDC - 1))
                nc.vector.tensor_copy(k_sb[:, sl], kacc[0:HD, 0:QG])

            def emit_v(h):
                """V^T projection + crossbar transpose to key-major chunks."""
                sl = slice(h * QG, (h + 1) * QG)
                nc.gpsimd.memset(vt_hd[HD:VTP, sl], 0.0)
                nc.gpsimd.memset(vt_hd[HD:HD + 1, sl], 1.0)
                vacc = oq_pool.tile([P, EMBED], f32, tag="oq", name="vacc")
                for c in range(NDC):
                    nc.tensor.matmul(vacc[0:HD, 0:QG],
                                     win_sb[:, c, 2 * HD:3 * HD],
                                     qts(c, h * QG, (h + 1) * QG),
                                     start=(c == 0), stop=(c == NDC - 1))
                nc.vector.tensor_copy(vt_hd[0:HD, sl], vacc[0:HD, 0:QG])
                (nc.sync if h % 2 == 0 else nc.scalar).dma_start_transpose(
                    vtr[h], vt_hd[:, sl])

            def emit_q(g, qacc, c):
                nc.tensor.matmul(qacc[0:HD, 0:QG], win_sb[:, c, 0:HD],
                                 qts(c, g * QG, (g + 1) * QG),
                                 start=(c == 0), stop=(c == NDC - 1))

            def emit_d(t, pool=None, tag="oq", normalize=True, q=None):
                # out_proj for seq rows [t*128, (t+1)*128) + normalization
                o_ps = (pool or oq_pool).tile([P, EMBED], f32, tag=tag,
                                              name="o_ps")
                nc.tensor.matmul(o_ps, ot_sb[0:HD, t * P:(t + 1) * P], wo_sb,
                                 start=True, stop=True)
                o_sb = o_pool.tile([P, EMBED], f16, tag="o", name="o_sb")
                if normalize:
                    nc.vector.tensor_scalar_mul(o_sb, o_ps,
                                                recip_all[:, t:t + 1])
                else:
                    # last group: host divides by the shipped denominators,
                    # keeping the recip path off the kernel's tail
                    nc.vector.tensor_copy(o_sb, o_ps)
                (q or (nc.sync if t % 2 == 0 else nc.scalar)).dma_start(
                    out=out_p[t * P:(t + 1) * P, :], in_=o_sb)

            def emit_evicts(g):
                # denominator row first: it gates recip -> out-proj norm
                gsl = slice(g * QG, (g + 1) * QG)
                nc.vector.tensor_copy(ot_sb[HD:HD + 1, gsl],
                                      accs[g][HD:HD + 1, :])
                if g == NQG - 1:
                    # ship raw denominators; host normalizes this group
                    nc.gpsimd.dma_start(out=den7[:, :],
                                        in_=ot_sb[HD:HD + 1, gsl])
                else:
                    for i in range(4):
                        t = 4 * g + i
                        den_qs[i].dma_start(
                            out=den_all[:, t:t + 1],
                            in_=ot_sb[HD:HD + 1, t * P:(t + 1) * P])
                    nc.vector.reciprocal(recip_all[:, 4 * g:4 * g + 4],
                                         den_all[:, 4 * g:4 * g + 4])
                nc.vector.tensor_copy(ot_sb[0:HD, gsl], accs[g][0:HD, :])

            # just-in-time weave points inside group 0: K before the batch
            # that first scores against it, V (needed only by the lagged AV)
            # after it
            weave_k = {}
            weave_v = {0: [0]}
            for h in range(1, NQG):
                weave_k.setdefault(max(0, (4 * h) // 3 - 1), []).append(h)
                weave_v.setdefault(min((4 * h) // 3 + 1, NB - 1), []).append(h)

            accs = {}
            p_tiles = {}
            qacc = None

            emit_k(0)
            qacc0 = oq_pool.tile([P, EMBED], f32, tag="oq", name="qacc0")
            for c in range(NDC):
                emit_q(0, qacc0, c)
            nc.vector.tensor_scalar_add(q_sb[:, 0:QG], qacc0[0:HD, 0:QG],
                                        bq_sb)

            def emit_scores_exp(g, b):
                chunks = batches[b]
                nb = len(chunks)
                s_ps = s_pool.tile([P, EB * QG], f32, tag="sps", name="s_ps")
                for i, s in enumerate(chunks):
                    nc.tensor.matmul(
                        s_ps[:, i * QG:(i + 1) * QG],
                        k_sb[:, s * P:(s + 1) * P],
                        q_sb[:, g * QG:(g + 1) * QG],
                        start=True, stop=True)
                p_sb = p_pool.tile([P, EB * QG], f16, tag="p", name="p_sb")
                nc.scalar.activation(p_sb[:, :nb * QG], s_ps[:, :nb * QG],
                                     Exp, scale=SCALE)
                p_tiles[(g, b)] = p_sb

            def emit_av(g, b):
                if b == 0:
                    accs[g] = acc_pool.tile([HD + 1, QG], f32, tag="acc",
                                            name="out_acc")
                p_sb = p_tiles.pop((g, b))
                for i, s in enumerate(batches[b]):
                    h, j = s // 4, s % 4
                    nc.tensor.matmul(
                        accs[g], vtr[h][:, j, 0:HD + 1],
                        p_sb[:, i * QG:(i + 1) * QG],
                        start=(s == 0), stop=(s == NSK - 1))

            def post_av(pg, pb):
                # AV runs LAG steps behind scores/exp so a stalled AV (waiting
                # on exp) never head-of-line-blocks the next scores matmuls in
                # the in-order PE queue.
                emit_av(pg, pb)
                if pb == NB - 1:
                    emit_evicts(pg)
                if pg >= 1 and 0 <= pb <= 3:
                    emit_d(4 * (pg - 1) + pb)

            qaccs = {}
            LAG = 3
            steps = [(g, b) for g in range(NQG) for b in range(NB)]
            def emit_qh(h):
                qa = oq_pool.tile([P, EMBED], f32, tag="oq", name="qa")
                for c in range(NDC):
                    emit_q(h, qa, c)
                nc.vector.tensor_scalar_add(
                    q_sb[:, h * QG:(h + 1) * QG], qa[0:HD, 0:QG], bq_sb)

            for n, (g, b) in enumerate(steps):
                if g == 0:
                    for h in weave_k.get(b, []):
                        emit_k(h)
                        emit_qh(h)
                emit_scores_exp(g, b)
                if g == 0:
                    for h in weave_v.get(b, []):
                        emit_v(h)
                if n >= LAG:
                    post_av(*steps[n - LAG])
            for n in range(len(steps) - LAG, len(steps)):
                post_av(*steps[n])
            tail_qs = [nc.sync, nc.scalar, nc.gpsimd, nc.scalar]
            for i in range(4):
                # the scores pipeline is drained; reuse its PSUM slots so the
                # four final out-proj chunks double-buffer instead of
                # serializing on the single oq slot; host normalizes these rows
                emit_d(4 * (NQG - 1) + i, pool=s_pool, tag="sps",
                       normalize=False, q=tail_qs[i])


def _in_maps(query, in_proj_weight, in_proj_bias, out_proj_weight):
    q2d = np.asarray(query, dtype=np.float32).reshape(S, EMBED)
    qt = np.ascontiguousarray(q2d.T.astype(np.float16))
    w = np.asarray(in_proj_weight, dtype=np.float32)
    b = np.asarray(in_proj_bias, dtype=np.float32)
    wout = np.asarray(out_proj_weight, dtype=np.float32)
    maps = []
    for h in range(HEADS):
        hs = slice(h * HD, (h + 1) * HD)
        ks = slice(EMBED + h * HD, EMBED + (h + 1) * HD)
        vs = slice(2 * EMBED + h * HD, 2 * EMBED + (h + 1) * HD)
        w_in = np.concatenate([w[hs].T, w[ks].T, w[vs].T], axis=1)
        maps.append({
            "qt": qt,
            "w_in": np.ascontiguousarray(w_in.astype(np.float16)),
            "wo": np.ascontiguousarray(wout[:, hs].T.astype(np.float16)),
            "bq": np.ascontiguousarray(b[hs].reshape(HD, 1)),
        })
    return maps


def get_nc():
    if "nc" not in _compiled:
        _compiled["nc"] = _build()
    return _compiled["nc"]


def kernel(query, in_proj_weight, in_proj_bias, out_proj_weight, out_proj_bias):
    from concourse.bass_utils import run_bass_kernel_spmd

    nc = get_nc()
    maps = _in_maps(query, in_proj_weight, in_proj_bias, out_proj_weight)
    res = run_bass_kernel_spmd(nc, maps, core_ids=list(range(HEADS)))
    _last["res"] = res
    acc = np.zeros((S, EMBED), dtype=np.float32)
    g7 = S - QG
    for h in range(HEADS):
        part = res.results[h]["out_p"].astype(np.float32)
        # last query group ships unnormalized + its softmax denominators
        den = res.results[h]["den7"].astype(np.float32).reshape(QG, 1)
        part[g7:, :] /= den
        acc += part
    # V bias and out_proj bias fold into one additive vector (softmax weights
    # sum to 1); the K bias cancels inside softmax entirely.
    bias_vec = (np.asarray(out_proj_bias, dtype=np.float32)
                + np.asarray(out_proj_weight, dtype=np.float32)
                @ np.asarray(in_proj_bias, dtype=np.float32)[2 * EMBED:3 * EMBED])
    acc += bias_vec[None, :]
    return acc.reshape(np.asarray(query).shape).astype(np.float32)


# revision 37
# speedup vs baseline: 1.0124x; 1.0124x over previous
"""Multi-head self-attention (B=1, S=4096, D=512, H=8) on 8 trn2 NeuronCores.

Sharding: one head per core (head/tensor parallel). Each core computes its
head's Q/K/V projections from the full (transposed) query, runs attention
without materializing the full score matrix (streaming over key chunks,
softmax denominator via a ones-row in the transposed V), applies its slice
of out_proj fused with softmax normalization, and writes an unnormalized
partial [S, D] output (fp16). Host sums the 8 partials and adds the folded
bias.

All matmul operands are fp16 (1 PE cycle/row); accumulation stays f32 in
PSUM. Bias handling exploits softmax invariance: the K bias cancels inside
softmax, and the V bias (+out bias) folds into a host-side additive vector,
so only the Q bias is applied on device.

Schedule: one global software pipeline over (group, key-batch) steps with
scores+exp emitted one step ahead of the AV accumulation, so the ACT engine
(exp) never stalls at group boundaries. K/V projections are woven
just-in-time into group 0's batches (chasing the qt DMA); V is projected as
V^T (cheap N=512 matmuls) and transposed to key-major via the DMA crossbar
(off the PE). Q projection for group g+1 and the out-proj of group g-1 are
woven into group g's batches.
"""

import sys

sys.path.insert(0, "/opt/trn_rl_repo")

import numpy as np

EMBED = 512
HEADS = 8
HD = 64          # head dim
S = 4096         # sequence length
P = 128          # partitions
NSK = S // P     # 32 key chunks of 128
QG = 512         # query group width (matmul free dim)
NQG = S // QG    # 8 query groups
NDC = EMBED // P # 4 contraction chunks for projections
SCALE = HD ** -0.5
EB = 3           # key chunks per exp batch (PSUM banks per slot)
NB = (NSK + EB - 1) // EB
VTP = 80         # vt_hd partition count (HD + ones row, padded to 16)

_compiled = {}
_last = {}


def _build(n_cores=8):
    import concourse.bacc as bacc
    import concourse.mybir as mybir
    import concourse.tile as tile

    f32 = mybir.dt.float32
    f16 = mybir.dt.float16

    nc = bacc.Bacc("TRN2", target_bir_lowering=False, debug=False,
                   num_devices=n_cores)

    qt = nc.dram_tensor("qt", [EMBED, S], f16, kind="ExternalInput")
    w_in = nc.dram_tensor("w_in", [EMBED, 3 * HD], f16, kind="ExternalInput")
    wo = nc.dram_tensor("wo", [HD, EMBED], f16, kind="ExternalInput")
    bq = nc.dram_tensor("bq", [HD, 1], f32, kind="ExternalInput")
    out_p = nc.dram_tensor("out_p", [S, EMBED], f16, kind="ExternalOutput")
    den7 = nc.dram_tensor("den7", [1, QG], f16, kind="ExternalOutput")

    with tile.TileContext(nc) as tc:
        _emit(tc, nc, mybir, qt, w_in, wo, bq, out_p, den7, f32, f16)

    nc.compile()
    return nc


def _emit(tc, nc, mybir, qt, w_in, wo, bq, out_p, den7, f32, f16):
    from contextlib import ExitStack

    Exp = mybir.ActivationFunctionType.Exp

    with ExitStack() as ctx:
        singles = ctx.enter_context(tc.tile_pool(name="singles", bufs=1))

        # warm up the ACT exp table while DMAs run
        warm = singles.tile([1, 1], f32, tag="warm")
        nc.vector.memset(warm, 0.0)
        warm2 = singles.tile([1, 1], f32, tag="warm2")
        nc.scalar.activation(warm2, warm, Exp)

        # --- weights first on the sync/gpsimd queues (they gate every
        # projection matmul; the scalar/ACT sequencer is busy loading the exp
        # table early on), then qt group 0, then the rest of qt. Each group's
        # contraction chunks land on different queues so a group becomes
        # usable as early as possible ---
        dqs = [nc.sync, nc.scalar, nc.gpsimd]
        qt_sb = [[None] * NQG for _ in range(NDC)]

        def qt_tile(c, h):
            tl = singles.tile([P, QG], f16, tag=f"qt{c}_{h}", name=f"qt{c}_{h}")
            qt_sb[c][h] = tl
            return tl

        win_sb = singles.tile([P, NDC, 3 * HD], f16, tag="win")
        wo_sb = singles.tile([HD, EMBED], f16, tag="wo")
        bq_sb = singles.tile([HD, 1], f32, tag="bq")

        # pair each contraction chunk's weights with its qt slice, in the
        # order the first K/Q/V chains consume them; wo/bq (needed much
        # later) go last
        nc.sync.dma_start(out=win_sb[:, 0, :], in_=w_in[0:P, :])
        nc.sync.dma_start(out=qt_tile(0, 0), in_=qt[0:P, 0:QG])
        nc.gpsimd.dma_start(out=win_sb[:, 1, :], in_=w_in[P:2 * P, :])
        nc.gpsimd.dma_start(out=qt_tile(1, 0), in_=qt[P:2 * P, 0:QG])
        nc.sync.dma_start(out=win_sb[:, 2, :], in_=w_in[2 * P:3 * P, :])
        nc.gpsimd.dma_start(out=qt_tile(2, 0), in_=qt[2 * P:3 * P, 0:QG])
        nc.sync.dma_start(out=qt_tile(3, 0), in_=qt[3 * P:4 * P, 0:QG])
        nc.gpsimd.dma_start(out=win_sb[:, 3, :], in_=w_in[3 * P:4 * P, :])
        nc.gpsimd.dma_start(out=bq_sb, in_=bq[:, :])
        nc.sync.dma_start(out=wo_sb, in_=wo[:, :])

        # rest of qt; the scalar queue idles during the exp-table load, so it
        # leads the rotation for group 1 onward
        rot = [nc.scalar, nc.sync, nc.gpsimd]
        qi = 0
        for h in range(1, NQG):
            for c in range(NDC):
                rot[qi % len(rot)].dma_start(
                    out=qt_tile(c, h),
                    in_=qt[c * P:(c + 1) * P, h * QG:(h + 1) * QG])
                qi += 1

        def qts(c, col0, col1):
            h = col0 // QG
            assert col1 <= (h + 1) * QG
            return qt_sb[c][h][:, col0 - h * QG:col1 - h * QG]

        # persistent activations
        k_sb = singles.tile([HD, S], f16, tag="k")        # K^T per head
        q_sb = singles.tile([HD, S], f16, tag="q")        # Q^T per head
        ot_sb = singles.tile([HD + 1, S], f16, tag="ot")  # out^T + denom row
        vt_hd = singles.tile([VTP, S], f16, tag="vthd")   # V^T + ones row
        vtr = [singles.tile([P, 4, VTP], f16, tag=f"vtr{h}", name=f"vtr{h}")
               for h in range(NQG)]                       # V key-major chunks
        den_all = singles.tile([P, NSK], f16, tag="den")
        recip_all = singles.tile([P, NSK], f32, tag="recip")

        with ExitStack() as cctx:
            s_pool = cctx.enter_context(
                tc.tile_pool(name="s_pool", bufs=2, space="PSUM"))
            acc_pool = cctx.enter_context(
                tc.tile_pool(name="acc_pool", bufs=1, space="PSUM"))
            oq_pool = cctx.enter_context(
                tc.tile_pool(name="oq_pool", bufs=1, space="PSUM"))
            p_pool = cctx.enter_context(tc.tile_pool(name="p_pool", bufs=5))
            o_pool = cctx.enter_context(tc.tile_pool(name="o_pool", bufs=3))

            batches = [list(range(b * EB, min((b + 1) * EB, NSK)))
                       for b in range(NB)]
            den_qs = [nc.gpsimd, nc.sync, nc.scalar, nc.gpsimd]

            def emit_k(h):
                sl = slice(h * QG, (h + 1) * QG)
                kacc = oq_pool.tile([P, EMBED], f32, tag="oq", name="kacc")
                for c in range(NDC):
                    nc.tensor.matmul(kacc[0:HD, 0:QG], win_sb[:, c, HD:2 * HD],
                                     qts(c, h * QG, (h + 1) * QG),
                                     start=(c == 0), stop=(c == NDC - 1))
                nc.vector.tensor_copy(k_sb[:, sl], kacc[0:HD, 0:QG])

            def emit_v(h):
                """V^T projection + crossbar transpose to key-major chunks."""
                sl = slice(h * QG, (h + 1) * QG)
                nc.gpsimd.memset(vt_hd[HD:VTP, sl], 0.0)
                nc.gpsimd.memset(vt_hd[HD:HD + 1, sl], 1.0)
                vacc = oq_pool.tile([P, EMBED], f32, tag="oq", name="vacc")
                for c in range(NDC):
                    nc.tensor.matmul(vacc[0:HD, 0:QG],
                                     win_sb[:, c, 2 * HD:3 * HD],
                                     qts(c, h * QG, (h + 1) * QG),
                                     start=(c == 0), stop=(c == NDC - 1))
                nc.vector.tensor_copy(vt_hd[0:HD, sl], vacc[0:HD, 0:QG])
                (nc.sync if h % 2 == 0 else nc.scalar).dma_start_transpose(
                    vtr[h], vt_hd[:, sl])

            def emit_q(g, qacc, c):
                nc.tensor.matmul(qacc[0:HD, 0:QG], win_sb[:, c, 0:HD],
                                 qts(c, g * QG, (g + 1) * QG),
                                 start=(c == 0), stop=(c == NDC - 1))

            def emit_d(t, pool=None, tag="oq", normalize=True, q=None):
                # out_proj for seq rows [t*128, (t+1)*128) + normalization
                o_ps = (pool or oq_pool).tile([P, EMBED], f32, tag=tag,
                                              name="o_ps")
                nc.tensor.matmul(o_ps, ot_sb[0:HD, t * P:(t + 1) * P], wo_sb,
                                 start=True, stop=True)
                o_sb = o_pool.tile([P, EMBED], f16, tag="o", name="o_sb")
                if normalize:
                    nc.vector.tensor_scalar_mul(o_sb, o_ps,
                                                recip_all[:, t:t + 1])
                else:
                    # last group: host divides by the shipped denominators,
                    # keeping the recip path off the kernel's tail
                    nc.vector.tensor_copy(o_sb, o_ps)
                (q or (nc.sync if t % 2 == 0 else nc.scalar)).dma_start(
                    out=out_p[t * P:(t + 1) * P, :], in_=o_sb)

            def emit_evicts(g):
                # denominator row first: it gates recip -> out-proj norm
                gsl = slice(g * QG, (g + 1) * QG)
                nc.vector.tensor_copy(ot_sb[HD:HD + 1, gsl],
                                      accs[g][HD:HD + 1, :])
                if g == NQG - 1:
                    # ship raw denominators; host normalizes this group
                    nc.gpsimd.dma_start(out=den7[:, :],
                                        in_=ot_sb[HD:HD + 1, gsl])
                else:
                    for i in range(4):
                        t = 4 * g + i
                        den_qs[i].dma_start(
                            out=den_all[:, t:t + 1],
                            in_=ot_sb[HD:HD + 1, t * P:(t + 1) * P])
                    nc.vector.reciprocal(recip_all[:, 4 * g:4 * g + 4],
                                         den_all[:, 4 * g:4 * g + 4])
                nc.vector.tensor_copy(ot_sb[0:HD, gsl], accs[g][0:HD, :])

            # just-in-time weave points inside group 0: K before the batch
            # that first scores against it, V (needed only by the lagged AV)
            # after it
            weave_k = {}
            weave_v = {0: [0]}
            for h in range(1, NQG):
                weave_k.setdefault(max(0, (4 * h) // 3 - 1), []).append(h)
                weave_v.setdefault(min((4 * h) // 3 + 1, NB - 1), []).append(h)

            accs = {}
            p_tiles = {}
            qacc = None

            emit_k(0)
            qacc0 = oq_pool.tile([P, EMBED], f32, tag="oq", name="qacc0")
            for c in range(NDC):
                emit_q(0, qacc0, c)
            nc.vector.tensor_scalar_add(q_sb[:, 0:QG], qacc0[0:HD, 0:QG],
                                        bq_sb)

            def emit_scores_exp(g, b):
                chunks = batches[b]
                nb = len(chunks)
                s_ps = s_pool.tile([P, EB * QG], f32, tag="sps", name="s_ps")
                for i, s in enumerate(chunks):
                    nc.tensor.matmul(
                        s_ps[:, i * QG:(i + 1) * QG],
                        k_sb[:, s * P:(s + 1) * P],
                        q_sb[:, g * QG:(g + 1) * QG],
                        start=True, stop=True)
                p_sb = p_pool.tile([P, EB * QG], f16, tag="p", name="p_sb")
                nc.scalar.activation(p_sb[:, :nb * QG], s_ps[:, :nb * QG],
                                     Exp, scale=SCALE)
                p_tiles[(g, b)] = p_sb

            def emit_av(g, b):
                if b == 0:
                    accs[g] = acc_pool.tile([HD + 1, QG], f32, tag="acc",
                                            name="out_acc")
                p_sb = p_tiles.pop((g, b))
                for i, s in enumerate(batches[b]):
                    h, j = s // 4, s % 4
                    nc.tensor.matmul(
                        accs[g], vtr[h][:, j, 0:HD + 1],
                        p_sb[:, i * QG:(i + 1) * QG],
                        start=(s == 0), stop=(s == NSK - 1))

            def post_av(pg, pb):
                # AV runs LAG steps behind scores/exp so a stalled AV (waiting
                # on exp) never head-of-line-blocks the next scores matmuls in
                # the in-order PE queue.
                emit_av(pg, pb)
                if pb == NB - 1:
                    emit_evicts(pg)
                if pg >= 1 and 0 <= pb <= 3:
                    emit_d(4 * (pg - 1) + pb)
                if pg == 0 and pb == 6:
                    qacc = oq_pool.tile([P, EMBED], f32, tag="oq",
                                        name="qacc")
                    for c in range(NDC):
                        emit_q(1, qacc, c)
                    nc.vector.tensor_scalar_add(
                        q_sb[:, QG:2 * QG], qacc[0:HD, 0:QG], bq_sb)
                if pg >= 1 and pg < NQG - 1:
                    if pb == 4:
                        qaccs[pg] = oq_pool.tile([P, EMBED], f32, tag="oq",
                                                 name="qacc")
                    if pb in (4, 5):
                        emit_q(pg + 1, qaccs[pg], pb - 4)
                    if pb == 6:
                        emit_q(pg + 1, qaccs[pg], 2)
                        emit_q(pg + 1, qaccs[pg], 3)
                    if pb == 7:
                        nc.vector.tensor_scalar_add(
                            q_sb[:, (pg + 1) * QG:(pg + 2) * QG],
                            qaccs[pg][0:HD, 0:QG], bq_sb)

            qaccs = {}
            LAG = 3
            steps = [(g, b) for g in range(NQG) for b in range(NB)]
            for n, (g, b) in enumerate(steps):
                if g == 0:
                    for h in weave_k.get(b, []):
                        emit_k(h)
                emit_scores_exp(g, b)
                if g == 0:
                    for h in weave_v.get(b, []):
                        emit_v(h)
                if n >= LAG:
                    post_av(*steps[n - LAG])
            for n in range(len(steps) - LAG, len(steps)):
                post_av(*steps[n])
            tail_qs = [nc.sync, nc.scalar, nc.gpsimd, nc.scalar]
            for i in range(4):
                # the scores pipeline is drained; reuse its PSUM slots so the
                # four final out-proj chunks double-buffer instead of
                # serializing on the single oq slot; host normalizes these rows
                emit_d(4 * (NQG - 1) + i, pool=s_pool, tag="sps",
                       normalize=False, q=tail_qs[i])


def _in_maps(query, in_proj_weight, in_proj_bias, out_proj_weight):
    q2d = np.asarray(query, dtype=np.float32).reshape(S, EMBED)
    qt = np.ascontiguousarray(q2d.T.astype(np.float16))
    w = np.asarray(in_proj_weight, dtype=np.float32)
    b = np.asarray(in_proj_bias, dtype=np.float32)
    wout = np.asarray(out_proj_weight, dtype=np.float32)
    maps = []
    for h in range(HEADS):
        hs = slice(h * HD, (h + 1) * HD)
        ks = slice(EMBED + h * HD, EMBED + (h + 1) * HD)
        vs = slice(2 * EMBED + h * HD, 2 * EMBED + (h + 1) * HD)
        w_in = np.concatenate([w[hs].T, w[ks].T, w[vs].T], axis=1)
        maps.append({
            "qt": qt,
            "w_in": np.ascontiguousarray(w_in.astype(np.float16)),
            "wo": np.ascontiguousarray(wout[:, hs].T.astype(np.float16)),
            "bq": np.ascontiguousarray(b[hs].reshape(HD, 1)),
        })
    return maps


def get_nc():
    if "nc" not in _compiled:
        _compiled["nc"] = _build()
    return _compiled["nc"]


def kernel(query, in_proj_weight, in_proj_bias, out_proj_weight, out_proj_bias):
    from concourse.bass_utils import run_bass_kernel_spmd

    nc = get_nc()
    maps = _in_maps(query, in_proj_weight, in_proj_bias, out_proj_weight)
    res = run_bass_kernel_spmd(nc, maps, core_ids=list(range(HEADS)))
    _last["res"] = res
    acc = np.zeros((S, EMBED), dtype=np.float32)
    g7 = S - QG
    for h in range(HEADS):
        part = res.results[h]["out_p"].astype(np.float32)
        # last query group ships unnormalized + its softmax denominators
        den = res.results[h]["den7"].astype(np.float32).reshape(QG, 1)
        part[g7:, :] /= den
        acc += part
    # V bias and out_proj bias fold into one additive vector (softmax weights
    # sum to 1); the K bias cancels inside softmax entirely.
    bias_vec = (np.asarray(out_proj_bias, dtype=np.float32)
                + np.asarray(out_proj_weight, dtype=np.float32)
                @ np.asarray(in_proj_bias, dtype=np.float32)[2 * EMBED:3 * EMBED])
    acc += bias_vec[None, :]
    return acc.reshape(np.asarray(query).shape).astype(np.float32)


# revision 38
# speedup vs baseline: 1.0362x; 1.0235x over previous
"""Multi-head self-attention (B=1, S=4096, D=512, H=8) on 8 trn2 NeuronCores.

Sharding: one head per core (head/tensor parallel). Each core computes its
head's Q/K/V projections from the full (transposed) query, runs attention
without materializing the full score matrix (streaming over key chunks,
softmax denominator via a ones-row in the transposed V), applies its slice
of out_proj fused with softmax normalization, and writes an unnormalized
partial [S, D] output (fp16). Host sums the 8 partials and adds the folded
bias.

All matmul operands are fp16 (1 PE cycle/row); accumulation stays f32 in
PSUM. Bias handling exploits softmax invariance: the K bias cancels inside
softmax, and the V bias (+out bias) folds into a host-side additive vector,
so only the Q bias is applied on device.

Schedule: one global software pipeline over (group, key-batch) steps with
scores+exp emitted one step ahead of the AV accumulation, so the ACT engine
(exp) never stalls at group boundaries. K/V projections are woven
just-in-time into group 0's batches (chasing the qt DMA); V is projected as
V^T (cheap N=512 matmuls) and transposed to key-major via the DMA crossbar
(off the PE). Q projection for group g+1 and the out-proj of group g-1 are
woven into group g's batches.
"""

import sys

sys.path.insert(0, "/opt/trn_rl_repo")

import numpy as np

EMBED = 512
HEADS = 8
HD = 64          # head dim
S = 4096         # sequence length
P = 128          # partitions
NSK = S // P     # 32 key chunks of 128
QG = 512         # query group width (matmul free dim)
NQG = S // QG    # 8 query groups
NDC = EMBED // P # 4 contraction chunks for projections
SCALE = HD ** -0.5
EB = 3           # key chunks per exp batch (PSUM banks per slot)
NB = (NSK + EB - 1) // EB
VTP = 80         # vt_hd partition count (HD + ones row, padded to 16)

_compiled = {}
_last = {}


def _build(n_cores=8):
    import concourse.bacc as bacc
    import concourse.mybir as mybir
    import concourse.tile as tile

    f32 = mybir.dt.float32
    f16 = mybir.dt.float16

    nc = bacc.Bacc("TRN2", target_bir_lowering=False, debug=False,
                   num_devices=n_cores)

    qt = nc.dram_tensor("qt", [EMBED, S], f16, kind="ExternalInput")
    w_in = nc.dram_tensor("w_in", [EMBED, 3 * HD], f16, kind="ExternalInput")
    wo = nc.dram_tensor("wo", [HD, EMBED], f16, kind="ExternalInput")
    bq = nc.dram_tensor("bq", [HD, 1], f32, kind="ExternalInput")
    out_p = nc.dram_tensor("out_p", [S, EMBED], f16, kind="ExternalOutput")
    den7 = nc.dram_tensor("den7", [1, QG], f16, kind="ExternalOutput")

    with tile.TileContext(nc) as tc:
        _emit(tc, nc, mybir, qt, w_in, wo, bq, out_p, den7, f32, f16)

    nc.compile()
    return nc


def _emit(tc, nc, mybir, qt, w_in, wo, bq, out_p, den7, f32, f16):
    from contextlib import ExitStack

    Exp = mybir.ActivationFunctionType.Exp

    with ExitStack() as ctx:
        singles = ctx.enter_context(tc.tile_pool(name="singles", bufs=1))

        # warm up the ACT exp table while DMAs run
        warm = singles.tile([1, 1], f32, tag="warm")
        nc.vector.memset(warm, 0.0)
        warm2 = singles.tile([1, 1], f32, tag="warm2")
        nc.scalar.activation(warm2, warm, Exp)

        # --- weights first on the sync/gpsimd queues (they gate every
        # projection matmul; the scalar/ACT sequencer is busy loading the exp
        # table early on), then qt group 0, then the rest of qt. Each group's
        # contraction chunks land on different queues so a group becomes
        # usable as early as possible ---
        dqs = [nc.sync, nc.scalar, nc.gpsimd]
        qt_sb = [[None] * NQG for _ in range(NDC)]

        def qt_tile(c, h):
            tl = singles.tile([P, QG], f16, tag=f"qt{c}_{h}", name=f"qt{c}_{h}")
            qt_sb[c][h] = tl
            return tl

        win_sb = singles.tile([P, NDC, 3 * HD], f16, tag="win")
        wo_sb = singles.tile([HD, EMBED], f16, tag="wo")
        bq_sb = singles.tile([HD, 1], f32, tag="bq")

        # pair each contraction chunk's weights with its qt slice, in the
        # order the first K/Q/V chains consume them; wo/bq (needed much
        # later) go last
        nc.sync.dma_start(out=win_sb[:, 0, :], in_=w_in[0:P, :])
        nc.sync.dma_start(out=qt_tile(0, 0), in_=qt[0:P, 0:QG])
        nc.gpsimd.dma_start(out=win_sb[:, 1, :], in_=w_in[P:2 * P, :])
        nc.gpsimd.dma_start(out=qt_tile(1, 0), in_=qt[P:2 * P, 0:QG])
        nc.sync.dma_start(out=win_sb[:, 2, :], in_=w_in[2 * P:3 * P, :])
        nc.gpsimd.dma_start(out=qt_tile(2, 0), in_=qt[2 * P:3 * P, 0:QG])
        nc.sync.dma_start(out=qt_tile(3, 0), in_=qt[3 * P:4 * P, 0:QG])
        nc.gpsimd.dma_start(out=win_sb[:, 3, :], in_=w_in[3 * P:4 * P, :])
        nc.gpsimd.dma_start(out=bq_sb, in_=bq[:, :])
        nc.sync.dma_start(out=wo_sb, in_=wo[:, :])

        # rest of qt; the scalar queue idles during the exp-table load, so it
        # leads the rotation for group 1 onward
        rot = [nc.scalar, nc.sync, nc.gpsimd]
        qi = 0
        for h in range(1, NQG):
            for c in range(NDC):
                rot[qi % len(rot)].dma_start(
                    out=qt_tile(c, h),
                    in_=qt[c * P:(c + 1) * P, h * QG:(h + 1) * QG])
                qi += 1

        def qts(c, col0, col1):
            h = col0 // QG
            assert col1 <= (h + 1) * QG
            return qt_sb[c][h][:, col0 - h * QG:col1 - h * QG]

        # persistent activations
        k_sb = singles.tile([HD, S], f16, tag="k")        # K^T per head
        q_sb = singles.tile([HD, S], f16, tag="q")        # Q^T per head
        ot_sb = singles.tile([HD + 1, S], f16, tag="ot")  # out^T + denom row
        vt_hd = singles.tile([VTP, S], f16, tag="vthd")   # V^T + ones row
        vtr = [singles.tile([P, 4, VTP], f16, tag=f"vtr{h}", name=f"vtr{h}")
               for h in range(NQG)]                       # V key-major chunks
        den_all = singles.tile([P, NSK], f16, tag="den")
        recip_all = singles.tile([P, NSK], f32, tag="recip")

        with ExitStack() as cctx:
            s_pool = cctx.enter_context(
                tc.tile_pool(name="s_pool", bufs=2, space="PSUM"))
            acc_pool = cctx.enter_context(
                tc.tile_pool(name="acc_pool", bufs=1, space="PSUM"))
            oq_pool = cctx.enter_context(
                tc.tile_pool(name="oq_pool", bufs=1, space="PSUM"))
            p_pool = cctx.enter_context(tc.tile_pool(name="p_pool", bufs=5))
            o_pool = cctx.enter_context(tc.tile_pool(name="o_pool", bufs=3))

            batches = [list(range(b * EB, min((b + 1) * EB, NSK)))
                       for b in range(NB)]
            den_qs = [nc.gpsimd, nc.sync, nc.scalar, nc.gpsimd]

            def emit_k(h):
                sl = slice(h * QG, (h + 1) * QG)
                kacc = oq_pool.tile([P, EMBED], f32, tag="oq", name="kacc")
                for c in range(NDC):
                    nc.tensor.matmul(kacc[0:HD, 0:QG], win_sb[:, c, HD:2 * HD],
                                     qts(c, h * QG, (h + 1) * QG),
                                     start=(c == 0), stop=(c == NDC - 1))
                nc.vector.tensor_copy(k_sb[:, sl], kacc[0:HD, 0:QG])

            def emit_v(h):
                """V^T projection + crossbar transpose to key-major chunks."""
                sl = slice(h * QG, (h + 1) * QG)
                nc.gpsimd.memset(vt_hd[HD:VTP, sl], 0.0)
                nc.gpsimd.memset(vt_hd[HD:HD + 1, sl], 1.0)
                vacc = oq_pool.tile([P, EMBED], f32, tag="oq", name="vacc")
                for c in range(NDC):
                    nc.tensor.matmul(vacc[0:HD, 0:QG],
                                     win_sb[:, c, 2 * HD:3 * HD],
                                     qts(c, h * QG, (h + 1) * QG),
                                     start=(c == 0), stop=(c == NDC - 1))
                nc.vector.tensor_copy(vt_hd[0:HD, sl], vacc[0:HD, 0:QG])
                (nc.sync if h % 2 == 0 else nc.scalar).dma_start_transpose(
                    vtr[h], vt_hd[:, sl])

            def emit_q(g, qacc, c):
                nc.tensor.matmul(qacc[0:HD, 0:QG], win_sb[:, c, 0:HD],
                                 qts(c, g * QG, (g + 1) * QG),
                                 start=(c == 0), stop=(c == NDC - 1))

            def emit_d(t, pool=None, tag="oq", normalize=True, q=None):
                # out_proj for seq rows [t*128, (t+1)*128) + normalization
                o_ps = (pool or oq_pool).tile([P, EMBED], f32, tag=tag,
                                              name="o_ps")
                nc.tensor.matmul(o_ps, ot_sb[0:HD, t * P:(t + 1) * P], wo_sb,
                                 start=True, stop=True)
                o_sb = o_pool.tile([P, EMBED], f16, tag="o", name="o_sb")
                if normalize:
                    nc.vector.tensor_scalar_mul(o_sb, o_ps,
                                                recip_all[:, t:t + 1])
                else:
                    # last group: host divides by the shipped denominators,
                    # keeping the recip path off the kernel's tail
                    nc.vector.tensor_copy(o_sb, o_ps)
                (q or (nc.sync if t % 2 == 0 else nc.scalar)).dma_start(
                    out=out_p[t * P:(t + 1) * P, :], in_=o_sb)

            def emit_evicts(g):
                # denominator row first: it gates recip -> out-proj norm
                gsl = slice(g * QG, (g + 1) * QG)
                nc.vector.tensor_copy(ot_sb[HD:HD + 1, gsl],
                                      accs[g][HD:HD + 1, :])
                if g == NQG - 1:
                    # ship raw denominators; host normalizes this group
                    nc.gpsimd.dma_start(out=den7[:, :],
                                        in_=ot_sb[HD:HD + 1, gsl])
                else:
                    for i in range(4):
                        t = 4 * g + i
                        den_qs[i].dma_start(
                            out=den_all[:, t:t + 1],
                            in_=ot_sb[HD:HD + 1, t * P:(t + 1) * P])
                    nc.vector.reciprocal(recip_all[:, 4 * g:4 * g + 4],
                                         den_all[:, 4 * g:4 * g + 4])
                nc.vector.tensor_copy(ot_sb[0:HD, gsl], accs[g][0:HD, :])

            # just-in-time weave points inside group 0: K before the batch
            # that first scores against it, V (needed only by the lagged AV)
            # after it
            weave_k = {}
            weave_v = {0: [0]}
            for h in range(1, NQG):
                weave_k.setdefault(max(0, (4 * h) // 3 - 1), []).append(h)
                weave_v.setdefault(min((4 * h) // 3 + 1, NB - 1), []).append(h)

            accs = {}
            p_tiles = {}
            qacc = None

            emit_k(0)
            qacc0 = oq_pool.tile([P, EMBED], f32, tag="oq", name="qacc0")
            for c in range(NDC):
                emit_q(0, qacc0, c)
            nc.vector.tensor_scalar_add(q_sb[:, 0:QG], qacc0[0:HD, 0:QG],
                                        bq_sb)

            def emit_scores_exp(g, b):
                chunks = batches[b]
                nb = len(chunks)
                s_ps = s_pool.tile([P, EB * QG], f32, tag="sps", name="s_ps")
                for i, s in enumerate(chunks):
                    nc.tensor.matmul(
                        s_ps[:, i * QG:(i + 1) * QG],
                        k_sb[:, s * P:(s + 1) * P],
                        q_sb[:, g * QG:(g + 1) * QG],
                        start=True, stop=True)
                p_sb = p_pool.tile([P, EB * QG], f16, tag="p", name="p_sb")
                nc.scalar.activation(p_sb[:, :nb * QG], s_ps[:, :nb * QG],
                                     Exp, scale=SCALE)
                p_tiles[(g, b)] = p_sb

            def emit_av(g, b):
                if b == 0:
                    accs[g] = acc_pool.tile([HD + 1, QG], f32, tag="acc",
                                            name="out_acc")
                p_sb = p_tiles.pop((g, b))
                for i, s in enumerate(batches[b]):
                    h, j = s // 4, s % 4
                    nc.tensor.matmul(
                        accs[g], vtr[h][:, j, 0:HD + 1],
                        p_sb[:, i * QG:(i + 1) * QG],
                        start=(s == 0), stop=(s == NSK - 1))

            def post_av(pg, pb):
                # AV runs LAG steps behind scores/exp so a stalled AV (waiting
                # on exp) never head-of-line-blocks the next scores matmuls in
                # the in-order PE queue.
                emit_av(pg, pb)
                if pb == NB - 1:
                    emit_evicts(pg)
                if pg >= 1 and 4 <= pb <= 7:
                    emit_d(4 * (pg - 1) + pb - 4)
                if pg == 0 and pb == 6:
                    qacc = oq_pool.tile([P, EMBED], f32, tag="oq",
                                        name="qacc")
                    for c in range(NDC):
                        emit_q(1, qacc, c)
                    nc.vector.tensor_scalar_add(
                        q_sb[:, QG:2 * QG], qacc[0:HD, 0:QG], bq_sb)
                if pg >= 1 and pg < NQG - 1:
                    if pb == 0:
                        qaccs[pg] = oq_pool.tile([P, EMBED], f32, tag="oq",
                                                 name="qacc")
                    if pb in (0, 1):
                        emit_q(pg + 1, qaccs[pg], pb)
                    if pb == 2:
                        emit_q(pg + 1, qaccs[pg], 2)
                        emit_q(pg + 1, qaccs[pg], 3)
                    if pb == 3:
                        nc.vector.tensor_scalar_add(
                            q_sb[:, (pg + 1) * QG:(pg + 2) * QG],
                            qaccs[pg][0:HD, 0:QG], bq_sb)

            qaccs = {}
            LAG = 3
            steps = [(g, b) for g in range(NQG) for b in range(NB)]
            for n, (g, b) in enumerate(steps):
                if g == 0:
                    for h in weave_k.get(b, []):
                        emit_k(h)
                emit_scores_exp(g, b)
                if g == 0:
                    for h in weave_v.get(b, []):
                        emit_v(h)
                if n >= LAG:
                    post_av(*steps[n - LAG])
            for n in range(len(steps) - LAG, len(steps)):
                post_av(*steps[n])
            tail_qs = [nc.sync, nc.scalar, nc.gpsimd, nc.scalar]
            for i in range(4):
                # the scores pipeline is drained; reuse its PSUM slots so the
                # four final out-proj chunks double-buffer instead of
                # serializing on the single oq slot; host normalizes these rows
                emit_d(4 * (NQG - 1) + i, pool=s_pool, tag="sps",
                       normalize=False, q=tail_qs[i])


def _in_maps(query, in_proj_weight, in_proj_bias, out_proj_weight):
    q2d = np.asarray(query, dtype=np.float32).reshape(S, EMBED)
    qt = np.ascontiguousarray(q2d.T.astype(np.float16))
    w = np.asarray(in_proj_weight, dtype=np.float32)
    b = np.asarray(in_proj_bias, dtype=np.float32)
    wout = np.asarray(out_proj_weight, dtype=np.float32)
    maps = []
    for h in range(HEADS):
        hs = slice(h * HD, (h + 1) * HD)
        ks = slice(EMBED + h * HD, EMBED + (h + 1) * HD)
        vs = slice(2 * EMBED + h * HD, 2 * EMBED + (h + 1) * HD)
        w_in = np.concatenate([w[hs].T, w[ks].T, w[vs].T], axis=1)
        maps.append({
            "qt": qt,
            "w_in": np.ascontiguousarray(w_in.astype(np.float16)),
            "wo": np.ascontiguousarray(wout[:, hs].T.astype(np.float16)),
            "bq": np.ascontiguousarray(b[hs].reshape(HD, 1)),
        })
    return maps


def get_nc():
    if "nc" not in _compiled:
        _compiled["nc"] = _build()
    return _compiled["nc"]


def kernel(query, in_proj_weight, in_proj_bias, out_proj_weight, out_proj_bias):
    from concourse.bass_utils import run_bass_kernel_spmd

    nc = get_nc()
    maps = _in_maps(query, in_proj_weight, in_proj_bias, out_proj_weight)
    res = run_bass_kernel_spmd(nc, maps, core_ids=list(range(HEADS)))
    _last["res"] = res
    acc = np.zeros((S, EMBED), dtype=np.float32)
    g7 = S - QG
    for h in range(HEADS):
        part = res.results[h]["out_p"].astype(np.float32)
        # last query group ships unnormalized + its softmax denominators
        den = res.results[h]["den7"].astype(np.float32).reshape(QG, 1)
        part[g7:, :] /= den
        acc += part
    # V bias and out_proj bias fold into one additive vector (softmax weights
    # sum to 1); the K bias cancels inside softmax entirely.
    bias_vec = (np.asarray(out_proj_bias, dtype=np.float32)
                + np.asarray(out_proj_weight, dtype=np.float32)
                @ np.asarray(in_proj_bias, dtype=np.float32)[2 * EMBED:3 * EMBED])
    acc += bias_vec[None, :]
    return acc.reshape(np.asarray(query).shape).astype(np.float32)
